# revision 56
# baseline (speedup 1.0000x reference)
"""Multi-head attention (B=2, S=2048, D=1024, H=16) on 8 Trainium2 NeuronCores.

Sharding: tensor-parallel on heads (4 groups of 4 heads) x data-parallel on
batch (2) -> 8 cores. Each core computes QKV projections for its head slice,
attention for its 4 heads, and a partial output projection; the host sums the
4 partials per batch element (the tensor-parallel allreduce) and adds bo.

Key performance structure:
- Scores (contraction dk=64) run as PE row-tiled PAIRS: the two heads of a
  feature chunk live at partitions 0-63 / 64-127, so their score matmuls
  execute concurrently on the (0,0) and (64,0) 64x128 PE tiles (2x scores).
- exp is split between the ACT (scalar) engine (exact, table-based) and the
  DVE (vector) engine using a Schraudolph fp16 bit-trick (one tensor_scalar:
  y = bitcast_fp16(int16(s*A + B)), ~3% max err on offloaded k-chunks only;
  softmax normalization cancels most of it).
- Attention passes are (qb, head-pair, ns) with a 16-chunk k-loop; Q proj for
  the second s-half and the output projection are interleaved into the
  attention slots as tensor-engine filler (separate PSUM tag).
- A ones-column appended to V makes the attn@V matmul accumulate softmax
  denominators (row DK of the PSUM accumulator).
"""

import numpy as np

import concourse.bass as bass  # noqa: F401
import concourse.tile as tile
from concourse import bacc, mybir
from concourse.bass_utils import run_bass_kernel_spmd

D_MODEL = 1024
NUM_HEADS = 16
DK = 64
B, S = 2, 2048
N_CORES = 8
GROUPS = 4                 # head groups (tensor parallel)
GW = D_MODEL // GROUPS     # 256 features per group = 4 heads
HPG = GROUPS               # heads per group = 4

F32 = mybir.dt.float32
BF16 = mybir.dt.float16  # 16-bit matmul operand dtype
I16 = mybir.dt.int16
EXPF = mybir.ActivationFunctionType.Exp
MULT = mybir.AluOpType.mult
ADD = mybir.AluOpType.add

# fp16 Schraudolph exp: exp(0.125*s) ~= bitcast_f16(int16(s*SCH_A + SCH_B))
SCH_A = 184.6649652337873   # (1024/ln2) * 0.125
SCH_B = 15315.25            # 15*1024 - 44.75 (minimax constant)
DVE_KS = (1, 4, 7, 10, 13)  # k-chunks whose exp runs on the vector engine
# (none adjacent to the pass end, so pass-boundary normalize work on the
# vector queue never delays a critical exp)


def _adv(gen, n=1):
    if gen is None:
        return
    for _ in range(n):
        try:
            next(gen)
        except StopIteration:
            return


def _emit(nc, tc, ctx):
    P = 128
    xqT = nc.dram_tensor("xqT", [D_MODEL, S], BF16, kind="ExternalInput")
    xkT = nc.dram_tensor("xkT", [D_MODEL, S], BF16, kind="ExternalInput")
    xvT = nc.dram_tensor("xvT", [D_MODEL, S], BF16, kind="ExternalInput")
    # weights arrive pre-tiled [partition, contraction-chunk, cols] so the
    # loads are contiguous DMAs
    wqT = nc.dram_tensor("wqT", [P, 8, GW], BF16, kind="ExternalInput")
    wkT = nc.dram_tensor("wkT", [P, 8, GW], BF16, kind="ExternalInput")
    wvT = nc.dram_tensor("wvT", [P, 8, GW], BF16, kind="ExternalInput")
    woT = nc.dram_tensor("woT", [P, 2, D_MODEL], BF16, kind="ExternalInput")
    bq2 = nc.dram_tensor("bq2", [P, 2], F32, kind="ExternalInput")
    bk2 = nc.dram_tensor("bk2", [P, 2], F32, kind="ExternalInput")
    bvr = nc.dram_tensor("bvr", [1, GW], F32, kind="ExternalInput")
    out = nc.dram_tensor("out", [S, D_MODEL], BF16, kind="ExternalOutput")

    consts = ctx.enter_context(tc.tile_pool(name="consts", bufs=1))
    persist = ctx.enter_context(tc.tile_pool(name="persist", bufs=1))
    sx = ctx.enter_context(tc.tile_pool(name="stexp", bufs=8))
    nrm = ctx.enter_context(tc.tile_pool(name="nrm", bufs=3))
    outp = ctx.enter_context(tc.tile_pool(name="outp", bufs=4))
    # PSUM: psA 2x4KB (scores pair ring / prologue proj), psB 1x4KB (filler
    # proj + oproj units), psC 2x2KB (attn@V accumulators / V proj) = 16KB.
    psA = ctx.enter_context(tc.tile_pool(name="psA", bufs=2, space="PSUM"))
    psB = ctx.enter_context(tc.tile_pool(name="psB", bufs=1, space="PSUM"))
    psC = ctx.enter_context(tc.tile_pool(name="psC", bufs=2, space="PSUM"))

    # ---- constants / weights -------------------------------------------
    wq_sb = consts.tile([P, 8, GW], BF16)
    wk_sb = consts.tile([P, 8, GW], BF16)
    wv_sb = consts.tile([P, 8, GW], BF16)
    wo_sb = consts.tile([P, 2, D_MODEL], BF16)
    nc.sync.dma_start(wq_sb[:], wqT[:])
    bq_sb = consts.tile([P, 2], F32)
    bk_sb = consts.tile([P, 2], F32)
    nc.sync.dma_start(bq_sb[:], bq2[:])
    nc.sync.dma_start(bk_sb[:], bk2[:])
    bv_row = consts.tile([1, GW], F32)
    nc.sync.dma_start(bv_row[:], bvr[:])
    bvb = consts.tile([P, GW], F32)
    nc.gpsimd.partition_broadcast(bvb[:], bv_row[:])

    # persistent activations (QT doubles as O.T after attention)
    QTs = [persist.tile([P, S], BF16, name=f"QT{j}") for j in range(2)]
    # KT split by (feature chunk, s-half)
    KT4 = [[persist.tile([P, 1024], BF16, name=f"KT{j}_{hh}") for hh in range(2)]
           for j in range(2)]
    Vaugs = [persist.tile([P, 8, HPG, DK + 1], BF16, name=f"Vaug{v}")
             for v in range(2)]
    # x inputs staged whole: a handful of large DMAs instead of ~40 small
    # ones — the ~0.65us per-trigger queue cost was pacing the prologue
    xq0 = persist.tile([P, 8, 1024], BF16, name="xq0")
    xq1 = persist.tile([P, 8, 1024], BF16, name="xq1")
    xk_t = persist.tile([P, 8, S], BF16, name="xk_t")
    xv_t = persist.tile([P, 8, S], BF16, name="xv_t")
    ones_f32 = consts.tile([P, 8, HPG], F32)
    nc.vector.memset(ones_f32[:], 1.0)
    # warm the ACT exp table during the DMA-bound prologue
    warm = consts.tile([1, 1], F32)
    nc.scalar.activation(out=warm[:], in_=ones_f32[0:1, 0, 0:1], func=EXPF)
    for v in range(2):
        nc.vector.tensor_scalar_add(Vaugs[v][:, :, :, DK], ones_f32[:], 0.0)

    xqT_r = xqT[:].rearrange("(c p) s -> p c s", p=P)
    xkT_r = xkT[:].rearrange("(c p) s -> p c s", p=P)
    xvT_r = xvT[:].rearrange("(c p) s -> p c s", p=P)

    # ---- input staging: pipelined medium-grain DMAs ---------------------
    # Aggregate input streaming runs at ~266GB/s and queues compete for it,
    # so only first-needed data streams in the early window: xq0/xv on sync,
    # xk0 on gpsimd, wk/wv on scalar (scalar then stays DMA-free for exps).
    # Deferred data (xq1, xk1, wo - first read by attention fillers) queues
    # behind it per-queue, since each queue completes transfers in order.
    nc.scalar.dma_start(wk_sb[:], wkT[:])
    nc.scalar.dma_start(wv_sb[:], wvT[:])
    for c in range(4):
        nc.sync.dma_start(xq0[:, 2 * c:2 * c + 2, :],
                          xqT_r[:, 2 * c:2 * c + 2, 0:1024])
    for c in range(4):
        nc.gpsimd.dma_start(xk_t[:, 2 * c:2 * c + 2, 0:1024],
                            xkT_r[:, 2 * c:2 * c + 2, 0:1024])
    # V in per-s-chunk tiles: a single [P,8,1024] strided DMA monopolizes
    # its queue for ~19us of descriptor generation
    for g in range(16):
        nc.sync.dma_start(xv_t[:, :, g * P:(g + 1) * P],
                          xvT_r[:, :, g * P:(g + 1) * P])
    for c in range(4):
        nc.gpsimd.dma_start(xk_t[:, 2 * c:2 * c + 2, 1024:2048],
                            xkT_r[:, 2 * c:2 * c + 2, 1024:2048])
    nc.gpsimd.dma_start(wo_sb[:], woT[:])
    for c in range(4):
        nc.sync.dma_start(xq1[:, 2 * c:2 * c + 2, :],
                          xqT_r[:, 2 * c:2 * c + 2, 1024:2048])

    # ---- projection units: one [P,512] PSUM accumulation + bias evac ----
    # q: (j, sb, ns) -> QTs[j][:, sb*1024+ns*512 : ...]
    # k: (j, sb, ns) -> KT4[j][sb][:, ns*512 : ...]
    def proj_unit(name, x_t, w_sb, b_sb, j, sb, ns, pool, tag):
        ps = pool.tile([P, 512], F32, tag=tag, name=f"u{name}{j}{sb}{ns}")
        csl = slice(sb * 1024 + ns * 512, sb * 1024 + (ns + 1) * 512)
        xsl = csl if x_t.shape[2] > 1024 else slice(ns * 512, (ns + 1) * 512)
        for i in range(8):
            nc.tensor.matmul(
                ps[:, :], w_sb[:, i, j * P:(j + 1) * P], x_t[:, i, xsl],
                start=(i == 0), stop=(i == 7),
            )
            yield
        if name == "k":
            dst = KT4[j][sb][:, ns * 512:(ns + 1) * 512]
        else:
            dst = QTs[j][:, csl]
        nc.vector.tensor_scalar_add(dst, ps[:, :], b_sb[:, j:j + 1])
        yield
        yield  # spacing for psum-slot reuse

    def run_gen(g):
        for _ in g:
            pass

    # ---- serial prologue: ONLY what the first attention pass needs ------
    # (Q sb0 j0, K sb0 j0, all of V); everything else becomes attention
    # filler so the exp stream starts ~25us earlier
    for ns in range(2):
        run_gen(proj_unit("q", xq0, wq_sb, bq_sb, 0, 0, ns, psA, "psA"))
    for ns in range(2):
        run_gen(proj_unit("k", xk_t, wk_sb, bk_sb, 0, 0, ns, psA, "psA"))
    # V: natural layout, s on partitions
    for g_ss in range(16):
        pv = psC.tile([P, 512], F32, tag="psC", name=f"pv{g_ss}")
        for i in range(8):
            nc.tensor.matmul(
                pv[:, 0:GW], xv_t[:, i, g_ss * P:(g_ss + 1) * P],
                wv_sb[:, i, :],
                start=(i == 0), stop=(i == 7),
            )
        nc.vector.tensor_tensor(
            Vaugs[g_ss // 8][:, g_ss % 8, :, 0:DK],
            pv[:, 0:GW].rearrange("p (h d) -> p h d", h=HPG),
            bvb[:].rearrange("p (h d) -> p h d", h=HPG),
            ADD,
        )

    # ---- filler streams (interleaved into attention slots) --------------
    def oproj_units(scs, tags=("psB",)):
        for u, sc in enumerate(scs):
            tag = tags[u % len(tags)]
            pool = psA if tag == "psA" else psB
            pso = pool.tile([P, 2, 512], F32, tag=tag, name=f"pso{sc}")
            for hd in range(2):
                for ms in range(2):
                    nc.tensor.matmul(
                        pso[:, ms, :],
                        QTs[hd][:, sc * P:(sc + 1) * P],
                        wo_sb[:, hd, ms * 512:(ms + 1) * 512],
                        start=(hd == 0), stop=(hd == 1),
                    )
                    yield
            ot = outp.tile([P, 1024], BF16, tag="osb", name=f"ot{sc}")
            # evacuate half on each of scalar/vector: parallel, and the DVE
            # half stays short so queued DVE exps start on time
            nc.scalar.copy(out=ot[:, 0:512], in_=pso[:, 0, :])
            nc.vector.tensor_copy(out=ot[:, 512:1024], in_=pso[:, 1, :])
            yield
            # alternate store queues so the final drains overlap
            dq = nc.gpsimd if sc % 2 == 0 else nc.sync
            dq.dma_start(out[sc * P:(sc + 1) * P, :], ot[:])
            yield
            yield  # spacing for the psB reuse by the next unit

    def chain(*gens):
        for g in gens:
            yield from g

    def delay(n):
        for _ in range(n):
            yield

    # ---- phase 2: attention, (qb, head-pair jc, ns) passes --------------
    def attn_pass(qb, jc, ns, filler, rate=1):
        qsl = slice(qb * 1024 + ns * 512, qb * 1024 + (ns + 1) * 512)
        po = [psC.tile([P, 512], F32, tag="psC", name=f"po{qb}{jc}{ns}{hh}")
              for hh in range(2)]

        def av(k, st_t):
            for hh in range(2):
                nc.tensor.matmul(
                    po[hh][0:DK + 1, :],
                    Vaugs[k // 8][:, k % 8, 2 * jc + hh, :],
                    st_t[:, hh, :],
                    start=(k == 0), stop=(k == 15),
                )

        pend = []  # attn@V deferred TWO slots so even a late-starting DVE
        # exp never stalls the in-order tensor queue
        for k in range(16):
            pst = psA.tile([P, 2, 512], F32, tag="psA",
                           name=f"pst{qb}{jc}{ns}{k}")
            for hh in range(2):
                pr = DK * hh
                # heads pr=0 / pr=64 run concurrently on PE row tiles
                nc.tensor.matmul(
                    pst[:, hh, :],
                    KT4[jc][k // 8][pr:pr + DK, (k % 8) * P:(k % 8 + 1) * P],
                    QTs[jc][pr:pr + DK, qsl],
                    start=True, stop=True,
                )
            st_t = sx.tile([P, 2, 512], BF16, tag="stexp",
                           name=f"st{qb}{jc}{ns}{k}")
            # first pass stays on ACT only: the vector queue is still
            # draining prologue evacuation work at that point
            if k in DVE_KS and not (qb == 0 and jc == 0 and ns == 0):
                nc.vector.tensor_scalar(
                    st_t[:].bitcast(I16), pst[:], SCH_A, SCH_B, MULT, ADD)
            elif k >= 14:
                # split the last chunks' exp per head: subtile release lets
                # the NEXT pass's scores reuse this pst ring slot after half
                # the exp latency instead of the full N=1024 instruction
                for hh in range(2):
                    nc.scalar.activation(out=st_t[:, hh, :],
                                         in_=pst[:, hh, :], func=EXPF,
                                         scale=0.125)
            else:
                nc.scalar.activation(out=st_t[:], in_=pst[:], func=EXPF,
                                     scale=0.125)
            pend.append((k, st_t))
            if len(pend) > 3:
                av(*pend.pop(0))
            _adv(filler, rate)
        for p in pend:
            av(*p)
        # normalize: row DK of po holds softmax denominators
        for hh in range(2):
            pr = DK * hh
            dn = nrm.tile([1, 512], F32, tag="dn", name=f"dn{qb}{jc}{ns}{hh}")
            nc.vector.tensor_copy(out=dn[:], in_=po[hh][DK:DK + 1, :])
            bc = nrm.tile([DK, 512], F32, tag="bc", name=f"bc{qb}{jc}{ns}{hh}")
            nc.vector.reciprocal_approx_fast(bc[0:1, :], dn[:])
            nc.gpsimd.partition_broadcast(bc[:], bc[0:1, :])
            # write O.T for this (head, q-slice) into QT's now-dead region
            nc.vector.tensor_tensor(
                QTs[jc][pr:pr + DK, qsl], po[hh][0:DK, :], bc[:], MULT)

    # qb0 filler: remaining projections ordered by first-use slot
    # (K1j0 by slot 8 of pass 1; Q0j1/K0j1 by pass 2 = slot 16; K1j1 by
    # slot 24; Q sb1 by qb1; then the first output-projection quarter)
    def pu(name, x_t, w, b, j, sb, ns):
        return proj_unit(name, x_t, w, b, j, sb, ns, psB, "psB")

    # IMPORTANT: units must be EMITTED (python order) before the pass that
    # reads them — the tile framework cannot depend on a future write.
    # Deadline order, unit ready by slot s needs 9*pos <= emitted yields:
    f1 = chain(
        pu("k", xk_t, wk_sb, bk_sb, 0, 1, 0),   # KT[0][1] ns0: slot 8
        pu("k", xk_t, wk_sb, bk_sb, 0, 1, 1),   # KT[0][1] ns1: slot 12
        pu("q", xq0, wq_sb, bq_sb, 1, 0, 0),    # QT[1] sb0 ns0: pass 3
        pu("k", xk_t, wk_sb, bk_sb, 1, 0, 0),   # KT[1][0]: pass 3
        pu("k", xk_t, wk_sb, bk_sb, 1, 0, 1),
        pu("k", xk_t, wk_sb, bk_sb, 1, 1, 0),   # KT[1][1]: pass 3 slot 40
        pu("k", xk_t, wk_sb, bk_sb, 1, 1, 1),
        pu("q", xq0, wq_sb, bq_sb, 1, 0, 1),    # QT[1] sb0 ns1: pass 4
        pu("q", xq1, wq_sb, bq_sb, 0, 1, 0),    # qb1 pass 1 (jc0)
        pu("q", xq1, wq_sb, bq_sb, 0, 1, 1),
        oproj_units(range(0, 4)),
    )
    # ns-major pass order so the first half of the output projection can
    # start once both head-pairs of ns=0 are normalized
    for i, (ns, jc) in enumerate([(0, 0), (0, 1), (1, 0), (1, 1)]):
        attn_pass(0, jc, ns, f1, rate=3 if i == 0 else 2)
    _adv(f1, 200)  # drain any remainder
    # Q sb1 jc1 units moved here (first read by qb1's 2nd pass) to relieve
    # qb0's tensor congestion; pass 1 runs rate 2 to keep emission ahead
    f2 = chain(
        pu("q", xq1, wq_sb, bq_sb, 1, 1, 0),
        pu("q", xq1, wq_sb, bq_sb, 1, 1, 1),
        oproj_units(range(4, 8)),
        delay(4),
        oproj_units(range(8, 12)),
    )
    for i, (ns, jc) in enumerate([(0, 0), (0, 1), (1, 0), (1, 1)]):
        attn_pass(1, jc, ns, f2, rate=2 if i == 0 else 1)
    _adv(f2, 80)
    # tail: attention is done, so alternate the two 4KB PSUM tags to keep
    # two output-projection units in flight
    for _ in oproj_units(range(12, 16), tags=("psB", "psA")):
        pass


_prog_cache = {}


def _build_program():
    if "nc" not in _prog_cache:
        from contextlib import ExitStack
        nc = bacc.Bacc("TRN2", target_bir_lowering=False)
        with tile.TileContext(nc) as tc:
            with ExitStack() as ctx:
                _emit(nc, tc, ctx)
        nc.compile()
        _prog_cache["nc"] = nc
    return _prog_cache["nc"]


def make_in_maps(query, key, value, Wq, bq, Wk, bk, Wv, bv, Wo, bo):
    query, key, value = (np.asarray(t, np.float32) for t in (query, key, value))
    Wq, Wk, Wv, Wo = (np.asarray(t, np.float32) for t in (Wq, Wk, Wv, Wo))
    bq, bk, bv = (np.asarray(t, np.float32) for t in (bq, bk, bv))
    xT = {b: {} for b in range(B)}
    for b in range(B):
        xT[b]["q"] = np.ascontiguousarray(query[b].T).astype(np.float16)
        xT[b]["k"] = np.ascontiguousarray(key[b].T).astype(np.float16)
        xT[b]["v"] = np.ascontiguousarray(value[b].T).astype(np.float16)
    in_maps = []
    for c in range(N_CORES):
        b, g = divmod(c, GROUPS)
        gs = slice(g * GW, (g + 1) * GW)
        def tile3(w):  # [1024, GW] -> [128, 8, GW] (partition, chunk, cols)
            return np.ascontiguousarray(
                w.reshape(8, 128, -1).transpose(1, 0, 2)).astype(np.float16)
        in_maps.append({
            "xqT": xT[b]["q"], "xkT": xT[b]["k"], "xvT": xT[b]["v"],
            "wqT": tile3(Wq[gs, :].T),
            "wkT": tile3(Wk[gs, :].T),
            "wvT": tile3(Wv[gs, :].T),
            "woT": np.ascontiguousarray(
                Wo[:, gs].T.reshape(2, 128, D_MODEL).transpose(1, 0, 2)
            ).astype(np.float16),
            "bq2": np.ascontiguousarray(bq[gs].reshape(2, 128).T),
            "bk2": np.ascontiguousarray(bk[gs].reshape(2, 128).T),
            "bvr": np.ascontiguousarray(bv[gs].reshape(1, GW)),
        })
    return in_maps


def run_on_hw(in_maps, trace=False, **kw):
    nc = _build_program()
    return run_bass_kernel_spmd(nc, in_maps, core_ids=list(range(N_CORES)),
                                trace=trace, **kw)


def kernel(query, key, value, Wq, bq, Wk, bk, Wv, bv, Wo, bo):
    in_maps = make_in_maps(query, key, value, Wq, bq, Wk, bk, Wv, bv, Wo, bo)
    res = run_on_hw(in_maps)
    out = np.zeros((B, S, D_MODEL), np.float32)
    for c in range(N_CORES):
        out[c // GROUPS] += res.results[c]["out"].astype(np.float32)
    out += np.asarray(bo, np.float32)
    return out


if __name__ == "__main__":
    # self-check against a pure-numpy reference
    rng = np.random.default_rng(0)
    sc = 1.0 / np.sqrt(D_MODEL)
    inp = dict(
        query=rng.standard_normal((B, S, D_MODEL), np.float32),
        key=rng.standard_normal((B, S, D_MODEL), np.float32),
        value=rng.standard_normal((B, S, D_MODEL), np.float32),
        Wq=(rng.standard_normal((D_MODEL, D_MODEL)) * sc).astype(np.float32),
        bq=rng.standard_normal(D_MODEL).astype(np.float32) * 0.1,
        Wk=(rng.standard_normal((D_MODEL, D_MODEL)) * sc).astype(np.float32),
        bk=rng.standard_normal(D_MODEL).astype(np.float32) * 0.1,
        Wv=(rng.standard_normal((D_MODEL, D_MODEL)) * sc).astype(np.float32),
        bv=rng.standard_normal(D_MODEL).astype(np.float32) * 0.1,
        Wo=(rng.standard_normal((D_MODEL, D_MODEL)) * sc).astype(np.float32),
        bo=rng.standard_normal(D_MODEL).astype(np.float32) * 0.1,
    )

    def np_ref(query, key, value, Wq, bq, Wk, bk, Wv, bv, Wo, bo):
        q = query.astype(np.float64) @ Wq.T.astype(np.float64) + bq
        k = key.astype(np.float64) @ Wk.T.astype(np.float64) + bk
        v = value.astype(np.float64) @ Wv.T.astype(np.float64) + bv
        q = q.reshape(B, S, NUM_HEADS, DK).transpose(0, 2, 1, 3)
        k = k.reshape(B, S, NUM_HEADS, DK).transpose(0, 2, 1, 3)
        v = v.reshape(B, S, NUM_HEADS, DK).transpose(0, 2, 1, 3)
        sc_ = np.einsum("bhqd,bhkd->bhqk", q, k) / np.sqrt(DK)
        sc_ -= sc_.max(-1, keepdims=True)
        a = np.exp(sc_)
        a /= a.sum(-1, keepdims=True)
        o = np.einsum("bhqk,bhkd->bhqd", a, v)
        o = o.transpose(0, 2, 1, 3).reshape(B, S, D_MODEL)
        return o @ Wo.T.astype(np.float64) + bo

    exp = np_ref(**inp)
    got = kernel(**inp)
    scale = np.abs(exp).max()
    err = np.abs(got - exp)
    print(f"max abs err {err.max():.4e}  rel {err.max() / scale:.4e}  "
          f"mean rel {err.mean() / scale:.4e}")


# revision 58
# speedup vs baseline: 1.1500x; 1.1500x over previous
"""Multi-head attention (B=2, S=2048, D=1024, H=16) on 8 Trainium2 NeuronCores.

Sharding: tensor-parallel on heads (4 groups of 4 heads) x data-parallel on
batch (2) -> 8 cores. Each core computes QKV projections for its head slice,
attention for its 4 heads, and a partial output projection; the host sums the
4 partials per batch element (the tensor-parallel allreduce) and adds bo.

Key performance structure:
- Scores (contraction dk=64) run as PE row-tiled PAIRS: the two heads of a
  feature chunk live at partitions 0-63 / 64-127, so their score matmuls
  execute concurrently on the (0,0) and (64,0) 64x128 PE tiles (2x scores).
- exp is split between the ACT (scalar) engine (exact, table-based) and the
  DVE (vector) engine using a Schraudolph fp16 bit-trick (one tensor_scalar:
  y = bitcast_fp16(int16(s*A + B)), ~3% max err on offloaded k-chunks only;
  softmax normalization cancels most of it).
- Attention passes are (qb, head-pair, ns) with a 16-chunk k-loop; Q proj for
  the second s-half and the output projection are interleaved into the
  attention slots as tensor-engine filler (separate PSUM tag).
- A ones-column appended to V makes the attn@V matmul accumulate softmax
  denominators (row DK of the PSUM accumulator).
"""

import numpy as np

import concourse.bass as bass  # noqa: F401
import concourse.tile as tile
from concourse import bacc, mybir
from concourse.bass_utils import run_bass_kernel_spmd

D_MODEL = 1024
NUM_HEADS = 16
DK = 64
B, S = 2, 2048
N_CORES = 8
GROUPS = 4                 # head groups (tensor parallel)
GW = D_MODEL // GROUPS     # 256 features per group = 4 heads
HPG = GROUPS               # heads per group = 4

F32 = mybir.dt.float32
BF16 = mybir.dt.float16  # 16-bit matmul operand dtype
I16 = mybir.dt.int16
EXPF = mybir.ActivationFunctionType.Exp
MULT = mybir.AluOpType.mult
ADD = mybir.AluOpType.add

# fp16 Schraudolph exp: exp(0.125*s) ~= bitcast_f16(int16(s*SCH_A + SCH_B))
SCH_A = 184.6649652337873   # (1024/ln2) * 0.125
SCH_B = 15315.25            # 15*1024 - 44.75 (minimax constant)
DVE_KS = (1, 4, 7, 10, 13)  # k-chunks whose exp runs on the vector engine
# (none adjacent to the pass end, so pass-boundary normalize work on the
# vector queue never delays a critical exp)


def _adv(gen, n=1):
    if gen is None:
        return
    for _ in range(n):
        try:
            next(gen)
        except StopIteration:
            return


def _emit(nc, tc, ctx):
    P = 128
    xqT = nc.dram_tensor("xqT", [D_MODEL, S], BF16, kind="ExternalInput")
    xkT = nc.dram_tensor("xkT", [D_MODEL, S], BF16, kind="ExternalInput")
    xvT = nc.dram_tensor("xvT", [D_MODEL, S], BF16, kind="ExternalInput")
    # weights arrive pre-tiled [partition, contraction-chunk, cols] so the
    # loads are contiguous DMAs
    wqT = nc.dram_tensor("wqT", [P, 8, GW], BF16, kind="ExternalInput")
    wkT = nc.dram_tensor("wkT", [P, 8, GW], BF16, kind="ExternalInput")
    wvT = nc.dram_tensor("wvT", [P, 8, GW], BF16, kind="ExternalInput")
    woT = nc.dram_tensor("woT", [P, 2, D_MODEL], BF16, kind="ExternalInput")
    bq2 = nc.dram_tensor("bq2", [P, 2], F32, kind="ExternalInput")
    bk2 = nc.dram_tensor("bk2", [P, 2], F32, kind="ExternalInput")
    bvr = nc.dram_tensor("bvr", [1, GW], F32, kind="ExternalInput")
    out = nc.dram_tensor("out", [S, D_MODEL], BF16, kind="ExternalOutput")

    consts = ctx.enter_context(tc.tile_pool(name="consts", bufs=1))
    persist = ctx.enter_context(tc.tile_pool(name="persist", bufs=1))
    sx = ctx.enter_context(tc.tile_pool(name="stexp", bufs=8))
    nrm = ctx.enter_context(tc.tile_pool(name="nrm", bufs=3))
    outp = ctx.enter_context(tc.tile_pool(name="outp", bufs=4))
    # PSUM: psA 2x4KB (scores pair ring / prologue proj), psB 1x4KB (filler
    # proj + oproj units), psC 2x2KB (attn@V accumulators / V proj) = 16KB.
    psA = ctx.enter_context(tc.tile_pool(name="psA", bufs=2, space="PSUM"))
    psB = ctx.enter_context(tc.tile_pool(name="psB", bufs=1, space="PSUM"))
    psC = ctx.enter_context(tc.tile_pool(name="psC", bufs=2, space="PSUM"))

    # ---- constants / weights -------------------------------------------
    wq_sb = consts.tile([P, 8, GW], BF16)
    wk_sb = consts.tile([P, 8, GW], BF16)
    wv_sb = consts.tile([P, 8, GW], BF16)
    wo_sb = consts.tile([P, 2, D_MODEL], BF16)
    nc.sync.dma_start(wq_sb[:], wqT[:])
    bq_sb = consts.tile([P, 2], F32)
    bk_sb = consts.tile([P, 2], F32)
    nc.sync.dma_start(bq_sb[:], bq2[:])
    nc.sync.dma_start(bk_sb[:], bk2[:])
    bv_row = consts.tile([1, GW], F32)
    nc.sync.dma_start(bv_row[:], bvr[:])
    bvb = consts.tile([P, GW], F32)
    nc.gpsimd.partition_broadcast(bvb[:], bv_row[:])

    # persistent activations (QT doubles as O.T after attention)
    QTs = [persist.tile([P, S], BF16, name=f"QT{j}") for j in range(2)]
    # KT split by (feature chunk, s-half)
    KT4 = [[persist.tile([P, 1024], BF16, name=f"KT{j}_{hh}") for hh in range(2)]
           for j in range(2)]
    Vaugs = [persist.tile([P, 8, HPG, DK + 1], BF16, name=f"Vaug{v}")
             for v in range(2)]
    # x inputs staged whole: a handful of large DMAs instead of ~40 small
    # ones — the ~0.65us per-trigger queue cost was pacing the prologue
    xq0 = persist.tile([P, 8, 1024], BF16, name="xq0")
    xq1 = persist.tile([P, 8, 1024], BF16, name="xq1")
    xk_t = persist.tile([P, 8, S], BF16, name="xk_t")
    xv_t = persist.tile([P, 8, S], BF16, name="xv_t")
    ones_f32 = consts.tile([P, 8, HPG], F32)
    nc.vector.memset(ones_f32[:], 1.0)
    # warm the ACT exp table during the DMA-bound prologue
    warm = consts.tile([1, 1], F32)
    nc.scalar.activation(out=warm[:], in_=ones_f32[0:1, 0, 0:1], func=EXPF)
    for v in range(2):
        nc.vector.tensor_scalar_add(Vaugs[v][:, :, :, DK], ones_f32[:], 0.0)

    xqT_r = xqT[:].rearrange("(c p) s -> p c s", p=P)
    xkT_r = xkT[:].rearrange("(c p) s -> p c s", p=P)
    xvT_r = xvT[:].rearrange("(c p) s -> p c s", p=P)

    # ---- input staging: pipelined medium-grain DMAs ---------------------
    # Aggregate input streaming runs at ~266GB/s and queues compete for it,
    # so only first-needed data streams in the early window: xq0/xv on sync,
    # xk0 on gpsimd, wk/wv on scalar (scalar then stays DMA-free for exps).
    # Deferred data (xq1, xk1, wo - first read by attention fillers) queues
    # behind it per-queue, since each queue completes transfers in order.
    nc.scalar.dma_start(wk_sb[:], wkT[:])
    nc.scalar.dma_start(wv_sb[:], wvT[:])
    for c in range(4):
        nc.sync.dma_start(xq0[:, 2 * c:2 * c + 2, :],
                          xqT_r[:, 2 * c:2 * c + 2, 0:1024])
    for c in range(4):
        nc.gpsimd.dma_start(xk_t[:, 2 * c:2 * c + 2, 0:1024],
                            xkT_r[:, 2 * c:2 * c + 2, 0:1024])
    # V in per-s-chunk tiles: a single [P,8,1024] strided DMA monopolizes
    # its queue for ~19us of descriptor generation
    for g in range(16):
        nc.sync.dma_start(xv_t[:, :, g * P:(g + 1) * P],
                          xvT_r[:, :, g * P:(g + 1) * P])
    for c in range(4):
        nc.gpsimd.dma_start(xk_t[:, 2 * c:2 * c + 2, 1024:2048],
                            xkT_r[:, 2 * c:2 * c + 2, 1024:2048])
    nc.gpsimd.dma_start(wo_sb[:], woT[:])
    for c in range(4):
        nc.sync.dma_start(xq1[:, 2 * c:2 * c + 2, :],
                          xqT_r[:, 2 * c:2 * c + 2, 1024:2048])

    # ---- projection units: one [P,512] PSUM accumulation + bias evac ----
    # q: (j, sb, ns) -> QTs[j][:, sb*1024+ns*512 : ...]
    # k: (j, sb, ns) -> KT4[j][sb][:, ns*512 : ...]
    def proj_unit(name, x_t, w_sb, b_sb, j, sb, ns, pool, tag):
        ps = pool.tile([P, 512], F32, tag=tag, name=f"u{name}{j}{sb}{ns}")
        csl = slice(sb * 1024 + ns * 512, sb * 1024 + (ns + 1) * 512)
        xsl = csl if x_t.shape[2] > 1024 else slice(ns * 512, (ns + 1) * 512)
        for i in range(8):
            nc.tensor.matmul(
                ps[:, :], w_sb[:, i, j * P:(j + 1) * P], x_t[:, i, xsl],
                start=(i == 0), stop=(i == 7),
            )
            yield
        if name == "k":
            dst = KT4[j][sb][:, ns * 512:(ns + 1) * 512]
        else:
            dst = QTs[j][:, csl]
        nc.vector.tensor_scalar_add(dst, ps[:, :], b_sb[:, j:j + 1])
        yield
        yield  # spacing for psum-slot reuse

    def run_gen(g):
        for _ in g:
            pass

    # ---- serial prologue: ONLY what the first attention pass needs ------
    # (Q sb0 j0, K sb0 j0, all of V); everything else becomes attention
    # filler so the exp stream starts ~25us earlier
    for ns in range(2):
        run_gen(proj_unit("q", xq0, wq_sb, bq_sb, 0, 0, ns, psA, "psA"))
    for ns in range(2):
        run_gen(proj_unit("k", xk_t, wk_sb, bk_sb, 0, 0, ns, psA, "psA"))
    # V: natural layout, s on partitions
    for g_ss in range(16):
        pv = psC.tile([P, 512], F32, tag="psC", name=f"pv{g_ss}")
        for i in range(8):
            nc.tensor.matmul(
                pv[:, 0:GW], xv_t[:, i, g_ss * P:(g_ss + 1) * P],
                wv_sb[:, i, :],
                start=(i == 0), stop=(i == 7),
            )
        nc.vector.tensor_tensor(
            Vaugs[g_ss // 8][:, g_ss % 8, :, 0:DK],
            pv[:, 0:GW].rearrange("p (h d) -> p h d", h=HPG),
            bvb[:].rearrange("p (h d) -> p h d", h=HPG),
            ADD,
        )

    # ---- filler streams (interleaved into attention slots) --------------
    def oproj_units(scs, tags=("psB",)):
        for u, sc in enumerate(scs):
            tag = tags[u % len(tags)]
            pool = psA if tag == "psA" else psB
            pso = pool.tile([P, 2, 512], F32, tag=tag, name=f"pso{sc}")
            for hd in range(2):
                for ms in range(2):
                    nc.tensor.matmul(
                        pso[:, ms, :],
                        QTs[hd][:, sc * P:(sc + 1) * P],
                        wo_sb[:, hd, ms * 512:(ms + 1) * 512],
                        start=(hd == 0), stop=(hd == 1),
                    )
                    yield
            ot = outp.tile([P, 1024], BF16, tag="osb", name=f"ot{sc}")
            # evacuate half on each of scalar/vector: parallel, and the DVE
            # half stays short so queued DVE exps start on time
            nc.scalar.copy(out=ot[:, 0:512], in_=pso[:, 0, :])
            nc.vector.tensor_copy(out=ot[:, 512:1024], in_=pso[:, 1, :])
            yield
            # alternate store queues so the final drains overlap
            dq = nc.gpsimd if sc % 2 == 0 else nc.sync
            dq.dma_start(out[sc * P:(sc + 1) * P, :], ot[:])
            yield
            yield  # spacing for the psB reuse by the next unit

    def chain(*gens):
        for g in gens:
            yield from g

    def delay(n):
        for _ in range(n):
            yield

    # ---- phase 2: attention, (qb, head-pair jc, ns) passes --------------
    def attn_pass(qb, jc, ns, filler, rate=1):
        qsl = slice(qb * 1024 + ns * 512, qb * 1024 + (ns + 1) * 512)
        po = [psC.tile([P, 512], F32, tag="psC", name=f"po{qb}{jc}{ns}{hh}")
              for hh in range(2)]

        def av(k, st_t):
            for hh in range(2):
                nc.tensor.matmul(
                    po[hh][0:DK + 1, :],
                    Vaugs[k // 8][:, k % 8, 2 * jc + hh, :],
                    st_t[:, hh, :],
                    start=(k == 0), stop=(k == 15),
                )

        pend = []  # attn@V deferred TWO slots so even a late-starting DVE
        # exp never stalls the in-order tensor queue
        for k in range(16):
            pst = psA.tile([P, 2, 512], F32, tag="psA",
                           name=f"pst{qb}{jc}{ns}{k}")
            for hh in range(2):
                pr = DK * hh
                # heads pr=0 / pr=64 run concurrently on PE row tiles
                nc.tensor.matmul(
                    pst[:, hh, :],
                    KT4[jc][k // 8][pr:pr + DK, (k % 8) * P:(k % 8 + 1) * P],
                    QTs[jc][pr:pr + DK, qsl],
                    start=True, stop=True,
                )
            st_t = sx.tile([P, 2, 512], BF16, tag="stexp",
                           name=f"st{qb}{jc}{ns}{k}")
            # first pass stays on ACT only: the vector queue is still
            # draining prologue evacuation work at that point
            if k in DVE_KS and not (qb == 0 and jc == 0 and ns == 0):
                nc.vector.tensor_scalar(
                    st_t[:].bitcast(I16), pst[:], SCH_A, SCH_B, MULT, ADD)
            elif k >= 14:
                # split the last chunks' exp per head: subtile release lets
                # the NEXT pass's scores reuse this pst ring slot after half
                # the exp latency instead of the full N=1024 instruction
                for hh in range(2):
                    nc.scalar.activation(out=st_t[:, hh, :],
                                         in_=pst[:, hh, :], func=EXPF,
                                         scale=0.125)
            else:
                nc.scalar.activation(out=st_t[:], in_=pst[:], func=EXPF,
                                     scale=0.125)
            pend.append((k, st_t))
            if len(pend) > 3:
                av(*pend.pop(0))
            _adv(filler, rate)
        for p in pend:
            av(*p)
        # normalize: row DK of po holds softmax denominators
        for hh in range(2):
            pr = DK * hh
            dn = nrm.tile([1, 512], F32, tag="dn", name=f"dn{qb}{jc}{ns}{hh}")
            nc.vector.tensor_copy(out=dn[:], in_=po[hh][DK:DK + 1, :])
            bc = nrm.tile([DK, 512], F32, tag="bc", name=f"bc{qb}{jc}{ns}{hh}")
            nc.vector.reciprocal_approx_fast(bc[0:1, :], dn[:])
            nc.gpsimd.partition_broadcast(bc[:], bc[0:1, :])
            # write O.T for this (head, q-slice) into QT's now-dead region
            nc.vector.tensor_tensor(
                QTs[jc][pr:pr + DK, qsl], po[hh][0:DK, :], bc[:], MULT)

    # qb0 filler: remaining projections ordered by first-use slot
    # (K1j0 by slot 8 of pass 1; Q0j1/K0j1 by pass 2 = slot 16; K1j1 by
    # slot 24; Q sb1 by qb1; then the first output-projection quarter)
    def pu(name, x_t, w, b, j, sb, ns):
        return proj_unit(name, x_t, w, b, j, sb, ns, psB, "psB")

    # IMPORTANT: units must be EMITTED (python order) before the pass that
    # reads them — the tile framework cannot depend on a future write.
    # Deadline order, unit ready by slot s needs 9*pos <= emitted yields:
    f1 = chain(
        pu("k", xk_t, wk_sb, bk_sb, 0, 1, 0),   # KT[0][1] ns0: slot 8
        pu("k", xk_t, wk_sb, bk_sb, 0, 1, 1),   # KT[0][1] ns1: slot 12
        pu("q", xq0, wq_sb, bq_sb, 1, 0, 0),    # QT[1] sb0 ns0: pass 3
        pu("k", xk_t, wk_sb, bk_sb, 1, 0, 0),   # KT[1][0]: pass 3
        pu("k", xk_t, wk_sb, bk_sb, 1, 0, 1),
        pu("k", xk_t, wk_sb, bk_sb, 1, 1, 0),   # KT[1][1]: pass 3 slot 40
        pu("k", xk_t, wk_sb, bk_sb, 1, 1, 1),
        pu("q", xq0, wq_sb, bq_sb, 1, 0, 1),    # QT[1] sb0 ns1: pass 4
        pu("q", xq1, wq_sb, bq_sb, 0, 1, 0),    # qb1 pass 1 (jc0)
        pu("q", xq1, wq_sb, bq_sb, 0, 1, 1),
        oproj_units(range(0, 4)),
    )
    # ns-major pass order so the first half of the output projection can
    # start once both head-pairs of ns=0 are normalized
    for i, (ns, jc) in enumerate([(0, 0), (0, 1), (1, 0), (1, 1)]):
        attn_pass(0, jc, ns, f1, rate=3 if i == 0 else 2)
    _adv(f1, 200)  # drain any remainder
    # Q sb1 jc1 units moved here (first read by qb1's 2nd pass) to relieve
    # qb0's tensor congestion; pass 1 runs rate 2 to keep emission ahead
    f2 = chain(
        pu("q", xq1, wq_sb, bq_sb, 1, 1, 0),
        pu("q", xq1, wq_sb, bq_sb, 1, 1, 1),
        oproj_units(range(4, 8)),
        delay(4),
        oproj_units(range(8, 12)),
    )
    for i, (ns, jc) in enumerate([(0, 0), (0, 1), (1, 0), (1, 1)]):
        attn_pass(1, jc, ns, f2, rate=2 if i == 0 else 1)
    _adv(f2, 80)
    # tail: attention is done, so alternate the two 4KB PSUM tags to keep
    # two output-projection units in flight
    for _ in oproj_units(range(12, 16), tags=("psB", "psA")):
        pass


_prog_cache = {}


def _build_program():
    if "nc" not in _prog_cache:
        from contextlib import ExitStack
        nc = bacc.Bacc("TRN2", target_bir_lowering=False)
        with tile.TileContext(nc) as tc:
            with ExitStack() as ctx:
                _emit(nc, tc, ctx)
        nc.compile()
        _prog_cache["nc"] = nc
    return _prog_cache["nc"]


def make_in_maps(query, key, value, Wq, bq, Wk, bk, Wv, bv, Wo, bo):
    query, key, value = (np.asarray(t, np.float32) for t in (query, key, value))
    Wq, Wk, Wv, Wo = (np.asarray(t, np.float32) for t in (Wq, Wk, Wv, Wo))
    bq, bk, bv = (np.asarray(t, np.float32) for t in (bq, bk, bv))
    xT = {b: {} for b in range(B)}
    for b in range(B):
        xT[b]["q"] = np.ascontiguousarray(query[b].T).astype(np.float16)
        xT[b]["k"] = np.ascontiguousarray(key[b].T).astype(np.float16)
        xT[b]["v"] = np.ascontiguousarray(value[b].T).astype(np.float16)
    in_maps = []
    for c in range(N_CORES):
        b, g = divmod(c, GROUPS)
        gs = slice(g * GW, (g + 1) * GW)
        def tile3(w):  # [1024, GW] -> [128, 8, GW] (partition, chunk, cols)
            return np.ascontiguousarray(
                w.reshape(8, 128, -1).transpose(1, 0, 2)).astype(np.float16)
        in_maps.append({
            "xqT": xT[b]["q"], "xkT": xT[b]["k"], "xvT": xT[b]["v"],
            "wqT": tile3(Wq[gs, :].T),
            "wkT": tile3(Wk[gs, :].T),
            "wvT": tile3(Wv[gs, :].T),
            "woT": np.ascontiguousarray(
                Wo[:, gs].T.reshape(2, 128, D_MODEL).transpose(1, 0, 2)
            ).astype(np.float16),
            "bq2": np.ascontiguousarray(bq[gs].reshape(2, 128).T),
            "bk2": np.ascontiguousarray(bk[gs].reshape(2, 128).T),
            "bvr": np.ascontiguousarray(bv[gs].reshape(1, GW)),
        })
    return in_maps


def run_on_hw(in_maps, trace=False, **kw):
    nc = _build_program()
    return run_bass_kernel_spmd(nc, in_maps, core_ids=list(range(N_CORES)),
                                trace=trace, **kw)


def kernel(query, key, value, Wq, bq, Wk, bk, Wv, bv, Wo, bo):
    in_maps = make_in_maps(query, key, value, Wq, bq, Wk, bk, Wv, bv, Wo, bo)
    res = run_on_hw(in_maps)
    out = np.zeros((B, S, D_MODEL), np.float32)
    for c in range(N_CORES):
        out[c // GROUPS] += res.results[c]["out"].astype(np.float32)
    out += np.asarray(bo, np.float32)
    return out


if __name__ == "__main__":
    # self-check against a pure-numpy reference
    rng = np.random.default_rng(0)
    sc = 1.0 / np.sqrt(D_MODEL)
    inp = dict(
        query=rng.standard_normal((B, S, D_MODEL), np.float32),
        key=rng.standard_normal((B, S, D_MODEL), np.float32),
        value=rng.standard_normal((B, S, D_MODEL), np.float32),
        Wq=(rng.standard_normal((D_MODEL, D_MODEL)) * sc).astype(np.float32),
        bq=rng.standard_normal(D_MODEL).astype(np.float32) * 0.1,
        Wk=(rng.standard_normal((D_MODEL, D_MODEL)) * sc).astype(np.float32),
        bk=rng.standard_normal(D_MODEL).astype(np.float32) * 0.1,
        Wv=(rng.standard_normal((D_MODEL, D_MODEL)) * sc).astype(np.float32),
        bv=rng.standard_normal(D_MODEL).astype(np.float32) * 0.1,
        Wo=(rng.standard_normal((D_MODEL, D_MODEL)) * sc).astype(np.float32),
        bo=rng.standard_normal(D_MODEL).astype(np.float32) * 0.1,
    )

    def np_ref(query, key, value, Wq, bq, Wk, bk, Wv, bv, Wo, bo):
        q = query.astype(np.float64) @ Wq.T.astype(np.float64) + bq
        k = key.astype(np.float64) @ Wk.T.astype(np.float64) + bk
        v = value.astype(np.float64) @ Wv.T.astype(np.float64) + bv
        q = q.reshape(B, S, NUM_HEADS, DK).transpose(0, 2, 1, 3)
        k = k.reshape(B, S, NUM_HEADS, DK).transpose(0, 2, 1, 3)
        v = v.reshape(B, S, NUM_HEADS, DK).transpose(0, 2, 1, 3)
        sc_ = np.einsum("bhqd,bhkd->bhqk", q, k) / np.sqrt(DK)
        sc_ -= sc_.max(-1, keepdims=True)
        a = np.exp(sc_)
        a /= a.sum(-1, keepdims=True)
        o = np.einsum("bhqk,bhkd->bhqd", a, v)
        o = o.transpose(0, 2, 1, 3).reshape(B, S, D_MODEL)
        return o @ Wo.T.astype(np.float64) + bo

    exp = np_ref(**inp)
    got = kernel(**inp)
    scale = np.abs(exp).max()
    err = np.abs(got - exp)
    print(f"max abs err {err.max():.4e}  rel {err.max() / scale:.4e}  "
          f"mean rel {err.mean() / scale:.4e}")
